# revision 14
# baseline (speedup 1.0000x reference)
"""Bahdanau-style additive attention with coverage, on 8 trn2 NeuronCores.

Math (per batch b):
  qb[j]    = sum_k q[k] * W1[k, j] + b1[j]                  (k in [0, 2H))
  pre[l,j] = sum_k keys[l,k] * W1[2H+k, j] + cov[l]*W1[4H, j] + qb[j]
  h        = tanh(pre)
  scores   = h @ W2                   (L,)
  attn     = softmax(scores)          (no max-subtraction: |scores| <= ||W2||_1)
  ctx_keys = attn^T @ keys            (2H,)   [reassociated: attn^T(keys@Wr) = (attn^T keys)@Wr]
  context  = ctx_keys @ Wr            (H,)

Sharding: data-parallel over batch B=16 -> 2 batches per core, params replicated.
Matmuls run as float32r (TF32-like, 1 cyc/row at N=512); K=1 reshape matmuls
stay plain fp32 (fp32r has PSUM destination-pattern restrictions).
"""

import numpy as np

# Problem constants (hardcoded per harness contract).
B = 16
L = 2048
H = 512
KD = 2 * H  # 1024: feature dim of q/keys, also MLP hidden dim
NCORES = 8
B_LOC = B // NCORES  # 2
P = 128
LCHUNK = 512
N_LC = L // LCHUNK  # 4
N_KT = KD // P  # 8 k-tiles (contraction)
N_JT = KD // P  # 8 j-tiles (hidden)
N_HT = H // P  # 4 h-tiles (output)

_CACHE = {}
PROFILE = False  # set True (e.g. from test.py) to capture an NTFF trace
LAST_RESULT = None


def _build(repeat=1):
    import concourse.bass as bass
    import concourse.mybir as mybir
    import concourse.tile as tile
    from concourse import bacc
    from concourse.masks import make_identity

    f32 = mybir.dt.float32
    f32r = mybir.dt.float32r
    AF = mybir.ActivationFunctionType

    def r(ap):  # view fp32 data as fp32r for reduced-precision matmul
        return ap.bitcast(f32r)

    nc = bacc.Bacc("TRN2", target_bir_lowering=False, debug=False,
                   num_devices=NCORES)

    query_d = nc.dram_tensor("query", (B_LOC, 1, KD), f32, kind="ExternalInput")
    keys_d = nc.dram_tensor("keys", (B_LOC, L, KD), f32, kind="ExternalInput")
    cov_d = nc.dram_tensor("coverage", (B_LOC, L, 1), f32, kind="ExternalInput")
    W1_d = nc.dram_tensor("W1", (2 * KD + 1, KD), f32, kind="ExternalInput")
    b1_d = nc.dram_tensor("b1", (KD,), f32, kind="ExternalInput")
    W2_d = nc.dram_tensor("W2", (KD, 1), f32, kind="ExternalInput")
    Wr_d = nc.dram_tensor("Wr", (KD, H), f32, kind="ExternalInput")
    ctx_d = nc.dram_tensor("context", (B_LOC, 1, H), f32, kind="ExternalOutput")
    attn_d = nc.dram_tensor("attention", (B_LOC, L, 1), f32, kind="ExternalOutput")

    with tile.TileContext(nc) as tc:
        with tc.tile_pool(name="singles", bufs=1) as singles, \
             tc.tile_pool(name="knat", bufs=2) as knat_pool, \
             tc.tile_pool(name="kT", bufs=2) as kT_pool, \
             tc.tile_pool(name="hT", bufs=2) as h_pool, \
             tc.tile_pool(name="small_sb", bufs=2) as small_pool, \
             tc.tile_pool(name="w1q", bufs=2) as w1q_pool, \
             tc.tile_pool(name="tp_psum", bufs=2, space="PSUM") as tp_psum, \
             tc.tile_pool(name="h_psum", bufs=2, space="PSUM") as h_psum, \
             tc.tile_pool(name="s_psum", bufs=1, space="PSUM") as s_psum, \
             tc.tile_pool(name="sm_psum", bufs=1, space="PSUM") as sm_psum, \
             tc.tile_pool(name="ctx_psum", bufs=2, space="PSUM") as ctx_psum:

            def load_keys_chunk(b, lc):
                keys_nat = knat_pool.tile([P, N_LC, KD], f32, tag="knat",
                                          name="keys_nat")
                for lt in range(N_LC):
                    nc.sync.dma_start(
                        r(keys_nat[:, lt]),
                        r(keys_d.ap()[b, lc * LCHUNK + lt * P:
                                      lc * LCHUNK + (lt + 1) * P, :]))
                return keys_nat

            # Chunk (0,0) keys DMA issued before anything else so the PE
            # pipeline head (transposes) is never starved behind param loads.
            first_keys = load_keys_chunk(0, 0)

            ident_f = singles.tile([P, P], f32)
            make_identity(nc, ident_f[:])
            ident = singles.tile([P, P], f32)
            nc.vector.tensor_copy(r(ident[:]), r(ident_f[:]))
            ones1 = singles.tile([1, 1], f32)
            nc.vector.memset(ones1[:], 1.0)

            # W1 keys-part, one tile per k-tile for fine-grained DMA deps
            w1k = []
            for kt in range(N_KT):
                w1k_t = singles.tile([P, KD], f32, name=f"w1k{kt}")
                nc.sync.dma_start(r(w1k_t[:]),
                                  r(W1_d.ap()[KD + kt * P:KD + (kt + 1) * P, :]))
                w1k.append(w1k_t)
            w1c = singles.tile([1, KD], f32)  # W1[2KD] (coverage row)
            nc.sync.dma_start(r(w1c[:]), r(W1_d.ap()[2 * KD:2 * KD + 1, :]))

            # b1, W2 transposed onto partitions: [p, t] = v[t*P + p]
            b1T = singles.tile([P, N_JT], f32)
            nc.sync.dma_start(b1T[:], b1_d.ap().rearrange("(t p) -> p t", p=P))
            W2T = singles.tile([P, N_JT], f32)
            nc.sync.dma_start(r(W2T[:]), r(W2_d.ap().rearrange("(t p) o -> p (t o)", p=P)))

            # q transposed: q_sb[p, b, kt] = query[b, 0, kt*P + p]
            q_sb = singles.tile([P, B_LOC, N_KT], f32)
            for b in range(B_LOC):
                nc.sync.dma_start(
                    r(q_sb[:, b]),
                    r(query_d.ap()[b, 0].rearrange("(t p) -> p t", p=P)))

            for _rep in range(repeat):
                def emit_qb():
                    # qbT[j-part, jt, b] = q @ W1q + b1. W1q streams through a
                    # small pool; all 8 jt groups accumulate into one PSUM bank.
                    qbT_ = singles.tile([P, N_JT, B_LOC], f32, name="qbT")
                    ps_qb = sm_psum.tile([P, N_JT * B_LOC], f32, tag="small",
                                         name="ps_qb")
                    for kt in range(N_KT):
                        w1q_c = w1q_pool.tile([P, KD], f32, tag="w1q",
                                              name="w1q_c")
                        nc.sync.dma_start(r(w1q_c[:]),
                                          r(W1_d.ap()[kt * P:(kt + 1) * P, :]))
                        for jt in range(N_JT):
                            nc.tensor.matmul(
                                ps_qb[:, jt * B_LOC:(jt + 1) * B_LOC],
                                r(w1q_c[:, jt * P:(jt + 1) * P]),
                                r(q_sb[:, :, kt]),
                                start=(kt == 0 and jt == 0),
                                stop=(kt == N_KT - 1 and jt == N_JT - 1))
                    for jt in range(N_JT):
                        nc.vector.tensor_scalar_add(
                            qbT_[:, jt], ps_qb[:, jt * B_LOC:(jt + 1) * B_LOC],
                            b1T[:, jt:jt + 1])
                    return qbT_

                # softmax state
                exp_row = singles.tile([1, B_LOC, L], f32, name="exp_row")
                sums = singles.tile([1, B_LOC, N_LC], f32, name="sums")
                rsum = singles.tile([1, B_LOC], f32, name="rsum")
                ctxT = singles.tile([P, N_KT, B_LOC], f32, name="ctxT")
                qbT = None
                wr = None

                for b in range(B_LOC):
                    cov = small_pool.tile([1, L], f32, tag="cov", bufs=1,
                                          name="cov")
                    nc.sync.dma_start(r(cov[:]), r(cov_d.ap()[b, :, 0].unsqueeze(0)))
                    ctx_ps = [ctx_psum.tile([1, LCHUNK], f32, tag="ctx",
                                            name=f"ctx{hf}")
                              for hf in range(2)]
                    for lc in range(N_LC):
                        if _rep == 0 and b == 0 and lc == 0:
                            keys_nat = first_keys
                        else:
                            keys_nat = load_keys_chunk(b, lc)
                        # transpose chunk -> keysT[k-part, kt, l]
                        keysT = kT_pool.tile([P, N_KT, LCHUNK], f32, tag="kT",
                                             name="keysT")
                        for kt in range(N_KT):
                            pst = tp_psum.tile([P, LCHUNK], f32, tag="tp",
                                               name="pst")
                            for lt in range(N_LC):
                                nc.tensor.transpose(
                                    r(pst[:, lt * P:(lt + 1) * P]),
                                    r(keys_nat[:, lt, kt * P:(kt + 1) * P]),
                                    r(ident[:]))
                            nc.vector.tensor_copy(r(keysT[:, kt]), r(pst[:]))
                        if qbT is None:
                            # emitted after chunk-0 transposes: the PE stream
                            # head isn't blocked on the W1q param stream
                            qbT = emit_qb()
                        # hT[j-part, jt, l] = tanh(W1k^T keysT + cov*w1c + qb)
                        hT = h_pool.tile([P, N_JT, LCHUNK], f32, tag="hT",
                                         name="hT")
                        for jt in range(N_JT):
                            ph = h_psum.tile([P, LCHUNK], f32, tag="h", name="ph")
                            for kt in range(N_KT):
                                nc.tensor.matmul(
                                    ph[:], r(w1k[kt][:, jt * P:(jt + 1) * P]),
                                    r(keysT[:, kt]), start=(kt == 0), stop=False)
                            nc.tensor.matmul(
                                ph[:], r(w1c[:, jt * P:(jt + 1) * P]),
                                r(cov[:, lc * LCHUNK:(lc + 1) * LCHUNK]),
                                start=False, stop=True)
                            nc.scalar.activation(r(hT[:, jt]), ph[:], AF.Tanh,
                                                 bias=qbT[:, jt, b:b + 1])
                        # scores for the chunk: (1, LCHUNK)
                        ps_s = s_psum.tile([1, LCHUNK], f32, tag="s", name="ps_s")
                        for jt in range(N_JT):
                            nc.tensor.matmul(ps_s[:], r(W2T[:, jt:jt + 1]),
                                             r(hT[:, jt]),
                                             start=(jt == 0), stop=(jt == N_JT - 1))
                        # exp (scores bounded by ||W2||_1: no max needed) + partial sum
                        nc.scalar.activation(
                            exp_row[:, b, lc * LCHUNK:(lc + 1) * LCHUNK], ps_s[:],
                            AF.Exp, accum_out=sums[:, b, lc:lc + 1])
                        # transpose exp chunk onto partitions (K=1 fp32 matmuls)
                        pse = sm_psum.tile([P, N_LC], f32, tag="small", name="pse")
                        for lt in range(N_LC):
                            nc.tensor.matmul(
                                pse[:, lt:lt + 1],
                                exp_row[:, b, lc * LCHUNK + lt * P:
                                        lc * LCHUNK + (lt + 1) * P],
                                ones1[:], start=True, stop=True)
                        expT = small_pool.tile([P, N_LC], f32, tag="expT",
                                               name="expT")
                        nc.vector.tensor_copy(r(expT[:]), pse[:])
                        # pass B: ctx_unnorm[k] += exp[l] * keys[l, k]
                        for lt in range(N_LC):
                            for hf in range(2):
                                nc.tensor.matmul(
                                    ctx_ps[hf][:], r(expT[:, lt:lt + 1]),
                                    r(keys_nat[:, lt, hf * LCHUNK:(hf + 1) * LCHUNK]),
                                    start=(lc == 0 and lt == 0),
                                    stop=(lc == N_LC - 1 and lt == N_LC - 1))
                        if _rep == 0 and b == 0 and lc == 0:
                            # Wr load deferred past the pipeline head
                            wr = singles.tile([P, N_KT, H], f32, name="wr")
                            for kt in range(N_KT):
                                nc.sync.dma_start(
                                    r(wr[:, kt]),
                                    r(Wr_d.ap()[kt * P:(kt + 1) * P, :]))
                        elif wr is None:
                            wr = singles.tile([P, N_KT, H], f32, name="wr")
                            for kt in range(N_KT):
                                nc.sync.dma_start(
                                    r(wr[:, kt]),
                                    r(Wr_d.ap()[kt * P:(kt + 1) * P, :]))

                    # ---- per-batch epilogue ----
                    nc.vector.tensor_reduce(rsum[:, b:b + 1], sums[:, b],
                                            axis=mybir.AxisListType.X,
                                            op=mybir.AluOpType.add)
                    nc.vector.reciprocal(rsum[:, b:b + 1], rsum[:, b:b + 1])
                    # attention out = exp * (1/sum)
                    attn_row = small_pool.tile([1, L], f32, tag="attn", bufs=1,
                                               name="attn_row")
                    nc.vector.tensor_scalar_mul(attn_row[:], exp_row[:, b],
                                                rsum[:, b:b + 1])
                    nc.sync.dma_start(attn_d.ap()[b, :, 0].unsqueeze(0), attn_row[:])
                    # ctx_keys row, normalized
                    ctx_row = small_pool.tile([1, KD], f32, tag="ctxrow", bufs=1,
                                              name="ctx_row")
                    for hf in range(2):
                        nc.scalar.mul(ctx_row[:, hf * LCHUNK:(hf + 1) * LCHUNK],
                                      ctx_ps[hf][:], rsum[:, b:b + 1])
                    # transpose ctx_keys onto partitions (K=1 fp32 matmuls)
                    psc = sm_psum.tile([P, N_KT], f32, tag="small", name="psc")
                    for kt in range(N_KT):
                        nc.tensor.matmul(psc[:, kt:kt + 1],
                                         ctx_row[:, kt * P:(kt + 1) * P],
                                         ones1[:], start=True, stop=True)
                    nc.vector.tensor_copy(r(ctxT[:, :, b]), psc[:])

                # ---------------- context = ctx_keys @ Wr (both batches) ----------------
                for ht in range(N_HT):
                    pf = sm_psum.tile([P, B_LOC], f32, tag="small", name="pf")
                    for kt in range(N_KT):
                        nc.tensor.matmul(pf[:], r(wr[:, kt, ht * P:(ht + 1) * P]),
                                         r(ctxT[:, kt, :]),
                                         start=(kt == 0), stop=(kt == N_KT - 1))
                    fin = small_pool.tile([P, B_LOC], f32, tag="fin", name="fin")
                    nc.vector.tensor_copy(fin[:], pf[:])
                    for b in range(B_LOC):
                        nc.sync.dma_start(
                            ctx_d.ap()[b, 0, ht * P:(ht + 1) * P].unsqueeze(-1),
                            fin[:, b:b + 1])

    nc.compile()
    return nc


def _get_nc(repeat=1):
    key = ("nc", repeat)
    if key not in _CACHE:
        _CACHE[key] = _build(repeat)
    return _CACHE[key]


def kernel(query, keys, coverage, W1, b1, W2, Wr):
    from concourse import bass_utils

    nc = _get_nc()
    query = np.ascontiguousarray(query, dtype=np.float32)
    keys = np.ascontiguousarray(keys, dtype=np.float32)
    coverage = np.ascontiguousarray(coverage, dtype=np.float32)
    W1 = np.ascontiguousarray(W1, dtype=np.float32)
    b1 = np.ascontiguousarray(b1, dtype=np.float32)
    W2 = np.ascontiguousarray(W2, dtype=np.float32)
    Wr = np.ascontiguousarray(Wr, dtype=np.float32)

    in_maps = []
    for c in range(NCORES):
        s = slice(c * B_LOC, (c + 1) * B_LOC)
        in_maps.append({
            "query": query[s], "keys": keys[s], "coverage": coverage[s],
            "W1": W1, "b1": b1, "W2": W2, "Wr": Wr,
        })
    res = bass_utils.run_bass_kernel_spmd(nc, in_maps, core_ids=list(range(NCORES)),
                                          trace=PROFILE)
    global LAST_RESULT
    LAST_RESULT = res
    context = np.concatenate([res.results[c]["context"] for c in range(NCORES)], axis=0)
    attention = np.concatenate([res.results[c]["attention"] for c in range(NCORES)], axis=0)
    return context, attention


# revision 17
# speedup vs baseline: 1.7383x; 1.7383x over previous
"""Bahdanau-style additive attention with coverage, on 8 trn2 NeuronCores.

Math (per batch b):
  qb[j]    = sum_k q[k] * W1[k, j] + b1[j]                  (k in [0, 2H))
  pre[l,j] = sum_k keys[l,k] * W1[2H+k, j] + cov[l]*W1[4H, j] + qb[j]
  h        = tanh(pre)
  scores   = h @ W2                   (L,)
  attn     = softmax(scores)          (no max-subtraction: |scores| <= ||W2||_1)
  ctx_keys = attn^T @ keys            (2H,)   [reassociated: attn^T(keys@Wr) = (attn^T keys)@Wr]
  context  = ctx_keys @ Wr            (H,)

Sharding: data-parallel over batch B=16 -> 2 batches per core, params replicated.
Matmuls run as float32r (TF32-like, 1 cyc/row at N=512); K=1 reshape matmuls
stay plain fp32 (fp32r has PSUM destination-pattern restrictions).
"""

import numpy as np

# Problem constants (hardcoded per harness contract).
B = 16
L = 2048
H = 512
KD = 2 * H  # 1024: feature dim of q/keys, also MLP hidden dim
NCORES = 8
B_LOC = B // NCORES  # 2
P = 128
LCHUNK = 512
N_LC = L // LCHUNK  # 4
N_KT = KD // P  # 8 k-tiles (contraction)
N_JT = KD // P  # 8 j-tiles (hidden)
N_HT = H // P  # 4 h-tiles (output)

_CACHE = {}
PROFILE = False  # set True (e.g. from test.py) to capture an NTFF trace
LAST_RESULT = None


def _build(repeat=1):
    import concourse.bass as bass
    import concourse.mybir as mybir
    import concourse.tile as tile
    from concourse import bacc
    from concourse.masks import make_identity

    f32 = mybir.dt.float32
    f32r = mybir.dt.float32r
    AF = mybir.ActivationFunctionType

    def r(ap):  # view fp32 data as fp32r for reduced-precision matmul
        return ap.bitcast(f32r)

    nc = bacc.Bacc("TRN2", target_bir_lowering=False, debug=False,
                   num_devices=NCORES)

    query_d = nc.dram_tensor("query", (B_LOC, 1, KD), f32, kind="ExternalInput")
    keys_d = nc.dram_tensor("keys", (B_LOC, L, KD), f32, kind="ExternalInput")
    cov_d = nc.dram_tensor("coverage", (B_LOC, L, 1), f32, kind="ExternalInput")
    W1_d = nc.dram_tensor("W1", (2 * KD + 1, KD), f32, kind="ExternalInput")
    b1_d = nc.dram_tensor("b1", (KD,), f32, kind="ExternalInput")
    W2_d = nc.dram_tensor("W2", (KD, 1), f32, kind="ExternalInput")
    Wr_d = nc.dram_tensor("Wr", (KD, H), f32, kind="ExternalInput")
    ctx_d = nc.dram_tensor("context", (B_LOC, 1, H), f32, kind="ExternalOutput")
    attn_d = nc.dram_tensor("attention", (B_LOC, L, 1), f32, kind="ExternalOutput")

    with tile.TileContext(nc) as tc:
        with tc.tile_pool(name="singles", bufs=1) as singles, \
             tc.tile_pool(name="knat", bufs=2) as knat_pool, \
             tc.tile_pool(name="kT", bufs=2) as kT_pool, \
             tc.tile_pool(name="hT", bufs=2) as h_pool, \
             tc.tile_pool(name="small_sb", bufs=2) as small_pool, \
             tc.tile_pool(name="w1q", bufs=2) as w1q_pool, \
             tc.tile_pool(name="tp_psum", bufs=2, space="PSUM") as tp_psum, \
             tc.tile_pool(name="h_psum", bufs=2, space="PSUM") as h_psum, \
             tc.tile_pool(name="s_psum", bufs=1, space="PSUM") as s_psum, \
             tc.tile_pool(name="sm_psum", bufs=1, space="PSUM") as sm_psum, \
             tc.tile_pool(name="ctx_psum", bufs=2, space="PSUM") as ctx_psum:

            def load_keys_chunk(b, lc):
                keys_nat = knat_pool.tile([P, N_LC, KD], f32, tag="knat",
                                          name="keys_nat")
                for lt in range(N_LC):
                    nc.sync.dma_start(
                        r(keys_nat[:, lt]),
                        r(keys_d.ap()[b, lc * LCHUNK + lt * P:
                                      lc * LCHUNK + (lt + 1) * P, :]))
                return keys_nat

            # Chunk (0,0) keys DMA issued before anything else so the PE
            # pipeline head (transposes) is never starved behind param loads.
            first_keys = load_keys_chunk(0, 0)

            ident_f = singles.tile([P, P], f32)
            make_identity(nc, ident_f[:])
            ident = singles.tile([P, P], f32)
            nc.vector.tensor_copy(r(ident[:]), r(ident_f[:]))
            ones1 = singles.tile([1, 1], f32)
            nc.vector.memset(ones1[:], 1.0)

            # ~5us of dummy matmuls: pulls the PE HAM clock-gate to 8/8
            # while the head DMAs stream, so real matmuls start warm.
            ps_warm = sm_psum.tile([P, P], f32, tag="small", name="ps_warm")
            for _w in range(44):
                nc.tensor.matmul(ps_warm[:], r(ident[:]), r(ident[:]),
                                 start=True, stop=True)

            # W1 keys-part, one tile per k-tile for fine-grained DMA deps
            w1k = []
            for kt in range(N_KT):
                w1k_t = singles.tile([P, KD], f32, name=f"w1k{kt}")
                nc.sync.dma_start(r(w1k_t[:]),
                                  r(W1_d.ap()[KD + kt * P:KD + (kt + 1) * P, :]))
                w1k.append(w1k_t)
            w1c = singles.tile([1, KD], f32)  # W1[2KD] (coverage row)
            nc.sync.dma_start(r(w1c[:]), r(W1_d.ap()[2 * KD:2 * KD + 1, :]))

            # b1, W2 transposed onto partitions: [p, t] = v[t*P + p]
            b1T = singles.tile([P, N_JT], f32)
            nc.sync.dma_start(b1T[:], b1_d.ap().rearrange("(t p) -> p t", p=P))
            W2T = singles.tile([P, N_JT], f32)
            nc.sync.dma_start(r(W2T[:]), r(W2_d.ap().rearrange("(t p) o -> p (t o)", p=P)))

            # q transposed: q_sb[p, b, kt] = query[b, 0, kt*P + p]
            q_sb = singles.tile([P, B_LOC, N_KT], f32)
            for b in range(B_LOC):
                nc.sync.dma_start(
                    r(q_sb[:, b]),
                    r(query_d.ap()[b, 0].rearrange("(t p) -> p t", p=P)))

            for _rep in range(repeat):
                def emit_qb():
                    # qbT[j-part, jt, b] = q @ W1q + b1. W1q streams through a
                    # small pool; all 8 jt groups accumulate into one PSUM bank.
                    qbT_ = singles.tile([P, N_JT, B_LOC], f32, name="qbT")
                    ps_qb = sm_psum.tile([P, N_JT * B_LOC], f32, tag="small",
                                         name="ps_qb")
                    for kt in range(N_KT):
                        w1q_c = w1q_pool.tile([P, KD], f32, tag="w1q",
                                              name="w1q_c")
                        nc.sync.dma_start(r(w1q_c[:]),
                                          r(W1_d.ap()[kt * P:(kt + 1) * P, :]))
                        for jt in range(N_JT):
                            nc.tensor.matmul(
                                ps_qb[:, jt * B_LOC:(jt + 1) * B_LOC],
                                r(w1q_c[:, jt * P:(jt + 1) * P]),
                                r(q_sb[:, :, kt]),
                                start=(kt == 0 and jt == 0),
                                stop=(kt == N_KT - 1 and jt == N_JT - 1))
                    for jt in range(N_JT):
                        nc.vector.tensor_scalar_add(
                            qbT_[:, jt], ps_qb[:, jt * B_LOC:(jt + 1) * B_LOC],
                            b1T[:, jt:jt + 1])
                    return qbT_

                def emit_transpose_group(keys_nat, keysT, kt):
                    pst = tp_psum.tile([P, LCHUNK], f32, tag="tp", name="pst")
                    for lt in range(N_LC):
                        nc.tensor.transpose(
                            r(pst[:, lt * P:(lt + 1) * P]),
                            r(keys_nat[:, lt, kt * P:(kt + 1) * P]),
                            r(ident[:]))
                    nc.vector.tensor_copy(r(keysT[:, kt]), r(pst[:]))

                # softmax state
                exp_row = singles.tile([1, B_LOC, L], f32, name="exp_row")
                sums = singles.tile([1, B_LOC, N_LC], f32, name="sums")
                rsum = singles.tile([1, B_LOC], f32, name="rsum")

                chunks = [(b, lc) for b in range(B_LOC) for lc in range(N_LC)]

                # ---- prologue: transpose chunk 0, then qb, then Wr load ----
                if _rep == 0:
                    keys_nat_cur = first_keys
                else:
                    keys_nat_cur = load_keys_chunk(0, 0)
                keysT_cur = kT_pool.tile([P, N_KT, LCHUNK], f32, tag="kT",
                                         name="keysT")
                for kt in range(N_KT):
                    emit_transpose_group(keys_nat_cur, keysT_cur, kt)
                qbT = emit_qb()
                wr = singles.tile([P, N_KT, H], f32, name="wr")
                for kt in range(N_KT):
                    nc.sync.dma_start(r(wr[:, kt]),
                                      r(Wr_d.ap()[kt * P:(kt + 1) * P, :]))

                cov = None
                ctx_ps = None
                for i, (b, lc) in enumerate(chunks):
                    if lc == 0:
                        cov = small_pool.tile([1, L], f32, tag="cov", bufs=1,
                                              name="cov")
                        nc.sync.dma_start(r(cov[:]),
                                          r(cov_d.ap()[b, :, 0].unsqueeze(0)))
                        ctx_ps = [ctx_psum.tile([1, LCHUNK], f32, tag="ctx",
                                                name=f"ctx{hf}")
                                  for hf in range(2)]
                    # prefetch + transpose the NEXT chunk, interleaved with
                    # this chunk's main matmuls (keeps the PE HAM-warm: no
                    # long transpose-only or idle windows)
                    nxt = chunks[i + 1] if i + 1 < len(chunks) else None
                    if nxt is not None:
                        keys_nat_nxt = load_keys_chunk(*nxt)
                        keysT_nxt = kT_pool.tile([P, N_KT, LCHUNK], f32,
                                                 tag="kT", name="keysT")
                    # hT[j-part, jt, l] = tanh(W1k^T keysT + cov*w1c + qb)
                    hT = h_pool.tile([P, N_JT, LCHUNK], f32, tag="hT", name="hT")
                    for jt in range(N_JT):
                        ph = h_psum.tile([P, LCHUNK], f32, tag="h", name="ph")
                        for kt in range(N_KT):
                            nc.tensor.matmul(
                                ph[:], r(w1k[kt][:, jt * P:(jt + 1) * P]),
                                r(keysT_cur[:, kt]), start=(kt == 0), stop=False)
                        nc.tensor.matmul(
                            ph[:], r(w1c[:, jt * P:(jt + 1) * P]),
                            r(cov[:, lc * LCHUNK:(lc + 1) * LCHUNK]),
                            start=False, stop=True)
                        nc.scalar.activation(r(hT[:, jt]), ph[:], AF.Tanh,
                                             bias=qbT[:, jt, b:b + 1])
                        if nxt is not None:
                            emit_transpose_group(keys_nat_nxt, keysT_nxt, jt)
                    # scores for the chunk: (1, LCHUNK)
                    ps_s = s_psum.tile([1, LCHUNK], f32, tag="s", name="ps_s")
                    for jt in range(N_JT):
                        nc.tensor.matmul(ps_s[:], r(W2T[:, jt:jt + 1]),
                                         r(hT[:, jt]),
                                         start=(jt == 0), stop=(jt == N_JT - 1))
                    # exp (scores bounded by ||W2||_1: no max needed) + partial sum
                    nc.scalar.activation(
                        exp_row[:, b, lc * LCHUNK:(lc + 1) * LCHUNK], ps_s[:],
                        AF.Exp, accum_out=sums[:, b, lc:lc + 1])
                    # transpose exp chunk onto partitions (K=1 fp32 matmuls)
                    pse = sm_psum.tile([P, N_LC], f32, tag="small", name="pse")
                    for lt in range(N_LC):
                        nc.tensor.matmul(
                            pse[:, lt:lt + 1],
                            exp_row[:, b, lc * LCHUNK + lt * P:
                                    lc * LCHUNK + (lt + 1) * P],
                            ones1[:], start=True, stop=True)
                    expT = small_pool.tile([P, N_LC], f32, tag="expT",
                                           name="expT")
                    nc.vector.tensor_copy(r(expT[:]), pse[:])
                    # pass B: ctx_unnorm[k] += exp[l] * keys[l, k]
                    for lt in range(N_LC):
                        for hf in range(2):
                            nc.tensor.matmul(
                                ctx_ps[hf][:], r(expT[:, lt:lt + 1]),
                                r(keys_nat_cur[:, lt, hf * LCHUNK:(hf + 1) * LCHUNK]),
                                start=(lc == 0 and lt == 0),
                                stop=(lc == N_LC - 1 and lt == N_LC - 1))
                    if nxt is not None:
                        keys_nat_cur = keys_nat_nxt
                        keysT_cur = keysT_nxt

                    if lc == N_LC - 1:
                        # ---- per-batch epilogue (batch 0's overlaps batch 1) ----
                        nc.vector.tensor_reduce(rsum[:, b:b + 1], sums[:, b],
                                                axis=mybir.AxisListType.X,
                                                op=mybir.AluOpType.add)
                        nc.vector.reciprocal(rsum[:, b:b + 1], rsum[:, b:b + 1])
                        # attention out = exp * (1/sum)
                        attn_row = small_pool.tile([1, L], f32, tag="attn",
                                                   bufs=1, name="attn_row")
                        nc.vector.tensor_scalar_mul(attn_row[:], exp_row[:, b],
                                                    rsum[:, b:b + 1])
                        nc.sync.dma_start(attn_d.ap()[b, :, 0].unsqueeze(0),
                                          attn_row[:])
                        # ctx_keys row, normalized
                        ctx_row = small_pool.tile([1, KD], f32, tag="ctxrow",
                                                  bufs=1, name="ctx_row")
                        for hf in range(2):
                            nc.scalar.mul(ctx_row[:, hf * LCHUNK:(hf + 1) * LCHUNK],
                                          ctx_ps[hf][:], rsum[:, b:b + 1])
                        # transpose ctx_keys onto partitions (K=1 fp32 matmuls)
                        psc = sm_psum.tile([P, N_KT], f32, tag="small", name="psc")
                        for kt in range(N_KT):
                            nc.tensor.matmul(psc[:, kt:kt + 1],
                                             ctx_row[:, kt * P:(kt + 1) * P],
                                             ones1[:], start=True, stop=True)
                        ctxT = small_pool.tile([P, N_KT], f32, tag="ctxT",
                                               bufs=2, name="ctxT")
                        nc.vector.tensor_copy(ctxT[:], psc[:])
                        # context[b] = ctx_keys @ Wr (plain fp32: N=1 rhs)
                        pf = sm_psum.tile([P, N_HT], f32, tag="small", name="pf")
                        for ht in range(N_HT):
                            for kt in range(N_KT):
                                nc.tensor.matmul(
                                    pf[:, ht:ht + 1],
                                    wr[:, kt, ht * P:(ht + 1) * P],
                                    ctxT[:, kt:kt + 1],
                                    start=(kt == 0), stop=(kt == N_KT - 1))
                        fin = small_pool.tile([P, N_HT], f32, tag="fin",
                                              name="fin")
                        nc.vector.tensor_copy(fin[:], pf[:])
                        nc.sync.dma_start(
                            ctx_d.ap()[b, 0].rearrange("(t p) -> p t", p=P),
                            fin[:])

    nc.compile()
    return nc


def _get_nc(repeat=1):
    key = ("nc", repeat)
    if key not in _CACHE:
        _CACHE[key] = _build(repeat)
    return _CACHE[key]


def kernel(query, keys, coverage, W1, b1, W2, Wr):
    from concourse import bass_utils

    nc = _get_nc()
    query = np.ascontiguousarray(query, dtype=np.float32)
    keys = np.ascontiguousarray(keys, dtype=np.float32)
    coverage = np.ascontiguousarray(coverage, dtype=np.float32)
    W1 = np.ascontiguousarray(W1, dtype=np.float32)
    b1 = np.ascontiguousarray(b1, dtype=np.float32)
    W2 = np.ascontiguousarray(W2, dtype=np.float32)
    Wr = np.ascontiguousarray(Wr, dtype=np.float32)

    in_maps = []
    for c in range(NCORES):
        s = slice(c * B_LOC, (c + 1) * B_LOC)
        in_maps.append({
            "query": query[s], "keys": keys[s], "coverage": coverage[s],
            "W1": W1, "b1": b1, "W2": W2, "Wr": Wr,
        })
    res = bass_utils.run_bass_kernel_spmd(nc, in_maps, core_ids=list(range(NCORES)),
                                          trace=PROFILE)
    global LAST_RESULT
    LAST_RESULT = res
    context = np.concatenate([res.results[c]["context"] for c in range(NCORES)], axis=0)
    attention = np.concatenate([res.results[c]["attention"] for c in range(NCORES)], axis=0)
    return context, attention


# revision 30
# speedup vs baseline: 1.9370x; 1.1143x over previous
"""Bahdanau-style additive attention with coverage, on 8 trn2 NeuronCores.

Math (per batch b):
  qb[j]    = sum_k q[k] * W1[k, j] + b1[j]                  (k in [0, 2H))
  pre[l,j] = sum_k keys[l,k] * W1[2H+k, j] + cov[l]*W1[4H, j] + qb[j]
  h        = tanh(pre)
  scores   = h @ W2                   (L,)
  attn     = softmax(scores)          (no max-subtraction: |scores| <= ||W2||_1)
  ctx_keys = attn^T @ keys            (2H,)   [reassociated: attn^T(keys@Wr) = (attn^T keys)@Wr]
  context  = ctx_keys @ Wr            (H,)

Sharding: data-parallel over batch B=16 -> 2 batches per core, params replicated.
Matmuls run as float32r (TF32-like, 1 cyc/row at N=512); K=1 reshape matmuls
stay plain fp32 (fp32r has PSUM destination-pattern restrictions).
"""

import numpy as np

# Problem constants (hardcoded per harness contract).
B = 16
L = 2048
H = 512
KD = 2 * H  # 1024: feature dim of q/keys, also MLP hidden dim
NCORES = 8
B_LOC = B // NCORES  # 2
P = 128
LCHUNK = 512
N_LC = L // LCHUNK  # 4
N_KT = KD // P  # 8 k-tiles (contraction)
N_JT = KD // P  # 8 j-tiles (hidden)
N_HT = H // P  # 4 h-tiles (output)

_CACHE = {}
PROFILE = False  # set True (e.g. from test.py) to capture an NTFF trace
LAST_RESULT = None


def _build(repeat=1):
    import concourse.bass as bass
    import concourse.mybir as mybir
    import concourse.tile as tile
    from concourse import bacc
    from concourse.masks import make_identity

    f32 = mybir.dt.float32
    bf16 = mybir.dt.bfloat16
    f32r = mybir.dt.float32r
    AF = mybir.ActivationFunctionType

    def r(ap):  # view fp32 data as fp32r for reduced-precision matmul
        return ap.bitcast(f32r)

    nc = bacc.Bacc("TRN2", target_bir_lowering=False, debug=False,
                   num_devices=NCORES)

    query_d = nc.dram_tensor("query", (B_LOC, 1, KD), f32, kind="ExternalInput")
    keys_d = nc.dram_tensor("keys", (B_LOC, L, KD), f32, kind="ExternalInput")
    cov_d = nc.dram_tensor("coverage", (B_LOC, L, 1), f32, kind="ExternalInput")
    W1_d = nc.dram_tensor("W1", (2 * KD + 1, KD), f32, kind="ExternalInput")
    b1_d = nc.dram_tensor("b1", (KD,), f32, kind="ExternalInput")
    W2_d = nc.dram_tensor("W2", (KD, 1), f32, kind="ExternalInput")
    Wr_d = nc.dram_tensor("Wr", (KD, H), f32, kind="ExternalInput")
    ctx_d = nc.dram_tensor("context", (B_LOC, 1, H), f32, kind="ExternalOutput")
    attn_d = nc.dram_tensor("attention", (B_LOC, L, 1), f32, kind="ExternalOutput")

    with tile.TileContext(nc) as tc:
        with tc.tile_pool(name="singles", bufs=1) as singles, \
             tc.tile_pool(name="knat", bufs=4) as knat_pool, \
             tc.tile_pool(name="kbf", bufs=12) as kbf_pool, \
             tc.tile_pool(name="kT", bufs=2) as kT_pool, \
             tc.tile_pool(name="hT", bufs=2) as h_pool, \
             tc.tile_pool(name="small_sb", bufs=2) as small_pool, \
             tc.tile_pool(name="w1q", bufs=4) as w1q_pool, \
             tc.tile_pool(name="tp_psum", bufs=2, space="PSUM") as tp_psum, \
             tc.tile_pool(name="h_psum", bufs=2, space="PSUM") as h_psum, \
             tc.tile_pool(name="s_psum", bufs=1, space="PSUM") as s_psum, \
             tc.tile_pool(name="sm_psum", bufs=1, space="PSUM") as sm_psum, \
             tc.tile_pool(name="ctx_psum", bufs=2, space="PSUM") as ctx_psum:

            def load_keys_chunk(b, lc):
                # one tile per 128-row slab: fine-grained DMA deps so each
                # transpose group can start as soon as its slab lands.
                # fp32 slab is cast to bf16 right away (matmul operand).
                tiles = []
                for lt in range(N_LC):
                    t = knat_pool.tile([P, KD], f32, tag="knat",
                                       name="keys_nat")
                    nc.sync.dma_start(
                        t[:],
                        keys_d.ap()[b, lc * LCHUNK + lt * P:
                                    lc * LCHUNK + (lt + 1) * P, :])
                    tb = kbf_pool.tile([P, KD], bf16, tag="kbf",
                                       name="keys_bf")
                    nc.vector.tensor_copy(tb[:], t[:])
                    tiles.append(tb)
                return tiles

            # Chunk (0,0) keys DMA issued before anything else so the PE
            # pipeline head (transposes) is never starved behind param loads.
            first_keys = load_keys_chunk(0, 0)

            ident_f = singles.tile([P, P], f32)
            make_identity(nc, ident_f[:])
            ident_b = singles.tile([P, P], bf16)  # 0/1 exact in bf16
            nc.vector.tensor_copy(ident_b[:], ident_f[:])
            ones1 = singles.tile([1, 1], f32)
            nc.vector.memset(ones1[:], 1.0)
            ones_row = singles.tile([1, P], f32)
            nc.vector.memset(ones_row[:], 1.0)

            # ~8us of dense dummy matmuls (bf16 N=512 streams, high PE duty):
            # pulls the PE HAM clock-gate to 8/8 while the head DMAs stream,
            # so the real matmuls start at 2.4GHz instead of 1.2.
            warm_sb = singles.tile([P, LCHUNK], bf16, name="warm_sb")
            nc.vector.memset(warm_sb[:], 0.125)
            ps_warm = s_psum.tile([P, LCHUNK], f32, tag="s", name="ps_warm")
            for _w in range(40):
                nc.tensor.matmul(ps_warm[:], ident_b[:], warm_sb[:],
                                 start=True, stop=True)

            # W1 keys-part in bf16: DMA fp32 staged through the knat pool,
            # cast once on the DVE. (DMAs issued inside rep 0, after the W1q
            # stream, so qb is never starved behind them.)
            w1k = []
            for kt in range(N_KT):
                w1k_t = singles.tile([P, KD], bf16, name=f"w1k{kt}")
                w1k.append(w1k_t)
            w1c_f = singles.tile([1, KD], f32)
            nc.sync.dma_start(w1c_f[:], W1_d.ap()[2 * KD:2 * KD + 1, :])
            w1c = singles.tile([1, KD], bf16)  # W1[2KD] (coverage row)
            nc.vector.tensor_copy(w1c[:], w1c_f[:])

            # b1, W2 transposed onto partitions: [p, t] = v[t*P + p]
            b1T = singles.tile([P, N_JT], f32)
            nc.sync.dma_start(b1T[:], b1_d.ap().rearrange("(t p) -> p t", p=P))
            W2T_f = singles.tile([P, N_JT], f32)
            nc.sync.dma_start(W2T_f[:], W2_d.ap().rearrange("(t p) o -> p (t o)", p=P))
            W2T = singles.tile([P, N_JT], bf16)
            nc.vector.tensor_copy(W2T[:], W2T_f[:])

            # q transposed: q_sb[p, b, kt] = query[b, 0, kt*P + p]
            q_sb = singles.tile([P, B_LOC, N_KT], f32)
            for b in range(B_LOC):
                nc.sync.dma_start(
                    r(q_sb[:, b]),
                    r(query_d.ap()[b, 0].rearrange("(t p) -> p t", p=P)))

            for _rep in range(repeat):
                def emit_qb():
                    # qbT[j-part, jt, b] = q @ W1q + b1. W1q streams through a
                    # small pool; all 8 jt groups accumulate into one PSUM bank.
                    qbT_ = singles.tile([P, N_JT, B_LOC], f32, name="qbT")
                    ps_qb = sm_psum.tile([P, N_JT * B_LOC], f32, tag="small",
                                         name="ps_qb")
                    for kt in range(N_KT):
                        w1q_c = w1q_pool.tile([P, KD], f32, tag="w1q",
                                              name="w1q_c")
                        for hf in range(2):
                            nc.sync.dma_start(
                                r(w1q_c[:, hf * LCHUNK:(hf + 1) * LCHUNK]),
                                r(W1_d.ap()[kt * P:(kt + 1) * P,
                                            hf * LCHUNK:(hf + 1) * LCHUNK]))
                        for jt in range(N_JT):
                            nc.tensor.matmul(
                                ps_qb[:, jt * B_LOC:(jt + 1) * B_LOC],
                                r(w1q_c[:, jt * P:(jt + 1) * P]),
                                r(q_sb[:, :, kt]),
                                start=(kt == 0 and jt == 0),
                                stop=(kt == N_KT - 1 and jt == N_JT - 1))
                        # keep the PE HAM-warm while the W1q stream trickles in
                        for _w in range(3):
                            nc.tensor.matmul(ps_warm[:], ident_b[:],
                                             warm_sb[:], start=True, stop=True)
                    for jt in range(N_JT):
                        nc.vector.tensor_scalar_add(
                            qbT_[:, jt], ps_qb[:, jt * B_LOC:(jt + 1) * B_LOC],
                            b1T[:, jt:jt + 1])
                    return qbT_

                def emit_transpose_group(keys_nat, keysT, g):
                    # group g handles slab lt = g//2, kt-half kh = g%2:
                    # 4 bf16 PE transposes into one PSUM bank + copy out.
                    # Transpose-mode doesn't register as PE activity for the
                    # HAM clock-gate, so a real (dummy) matmul follows each
                    # group: the gate never sees a full idle window and the
                    # PE stays at 2.4GHz.
                    lt, kh = g // 2, g % 2
                    pst = tp_psum.tile([P, LCHUNK], bf16, tag="tp", name="pst")
                    for j in range(4):
                        kt = kh * 4 + j
                        nc.tensor.transpose(
                            pst[:, j * P:(j + 1) * P],
                            keys_nat[lt][:, kt * P:(kt + 1) * P],
                            ident_b[:])
                    nc.vector.tensor_copy(
                        keysT[:, kh * 4:(kh + 1) * 4, lt * P:(lt + 1) * P],
                        pst[:].rearrange("p (j l) -> p j l", j=4))

                # softmax state
                sums = singles.tile([1, B_LOC, N_LC], f32, name="sums")
                rsum = singles.tile([1, B_LOC], f32, name="rsum")

                chunks = [(b, lc) for b in range(B_LOC) for lc in range(N_LC)]

                # ---- prologue: qb (its W1q DMAs go out first among params,
                # PE meanwhile runs the warmup), then transpose chunk 0 ----
                qbT = emit_qb()
                if _rep == 0:
                    for kt in range(N_KT):
                        w1k_f = knat_pool.tile([P, KD], f32, tag="knat",
                                               name="w1k_f")
                        nc.sync.dma_start(
                            w1k_f[:],
                            W1_d.ap()[KD + kt * P:KD + (kt + 1) * P, :])
                        nc.vector.tensor_copy(w1k[kt][:], w1k_f[:])
                    keys_nat_cur = first_keys
                else:
                    keys_nat_cur = load_keys_chunk(0, 0)
                keysT_cur = kT_pool.tile([P, N_KT, LCHUNK], bf16, tag="kT",
                                         name="keysT")
                for g in range(8):
                    emit_transpose_group(keys_nat_cur, keysT_cur, g)
                loaded = {1: load_keys_chunk(*[(b, lc) for b in range(B_LOC)
                                               for lc in range(N_LC)][1])}
                wr = singles.tile([P, N_KT, H], f32, name="wr")
                for kt in range(N_KT):
                    nc.sync.dma_start(wr[:, kt],
                                      Wr_d.ap()[kt * P:(kt + 1) * P, :])

                def emit_epilogue(b, ctx_ps, expT_all):
                    nc.vector.tensor_reduce(rsum[:, b:b + 1], sums[:, b],
                                            axis=mybir.AxisListType.X,
                                            op=mybir.AluOpType.add)
                    nc.vector.reciprocal(rsum[:, b:b + 1], rsum[:, b:b + 1])
                    # broadcast 1/sum over partitions via a K=1 matmul
                    ps_rb = sm_psum.tile([P, 1], f32, tag="small", name="ps_rb")
                    nc.tensor.matmul(ps_rb[:], ones_row[:], rsum[:, b:b + 1],
                                     start=True, stop=True)
                    rsum_b = small_pool.tile([P, 1], f32, tag="rsum_b",
                                             name="rsum_b")
                    nc.vector.tensor_copy(rsum_b[:], ps_rb[:])
                    # attention out: normalize the partition-major exp tile,
                    # transpose back to l-major on the PE, then one DMA
                    attn_n = small_pool.tile([P, 4 * N_LC], f32, tag="attn_n",
                                             name="attn_n")
                    nc.vector.tensor_scalar_mul(attn_n[:], expT_all[:],
                                                rsum_b[:])
                    ps_at = tp_psum.tile([4 * N_LC, P], f32, tag="tp",
                                         name="ps_at")
                    nc.tensor.transpose(ps_at[:], attn_n[:], ident_f[:])
                    attn_sb = small_pool.tile([4 * N_LC, P], f32, tag="attn_sb",
                                              name="attn_sb")
                    nc.vector.tensor_copy(attn_sb[:], ps_at[:])
                    nc.sync.dma_start(
                        attn_d.ap()[b, :, 0].rearrange("(t l) -> t l", l=P),
                        attn_sb[:])
                    # ctx_keys row, normalized
                    ctx_row = small_pool.tile([1, KD], f32, tag="ctxrow",
                                              bufs=1, name="ctx_row")
                    for hf in range(2):
                        nc.scalar.mul(ctx_row[:, hf * LCHUNK:(hf + 1) * LCHUNK],
                                      ctx_ps[hf][:], rsum[:, b:b + 1])
                    # transpose ctx_keys onto partitions (K=1 fp32 matmuls)
                    psc = sm_psum.tile([P, N_KT], f32, tag="small", name="psc")
                    for kt in range(N_KT):
                        nc.tensor.matmul(psc[:, kt:kt + 1],
                                         ctx_row[:, kt * P:(kt + 1) * P],
                                         ones1[:], start=True, stop=True)
                    ctxT = small_pool.tile([P, N_KT], f32, tag="ctxT",
                                           bufs=2, name="ctxT")
                    nc.vector.tensor_copy(ctxT[:], psc[:])
                    # context[b] = ctx_keys @ Wr (plain fp32: N=1 rhs)
                    pf = sm_psum.tile([P, N_HT], f32, tag="small", name="pf")
                    for ht in range(N_HT):
                        for kt in range(N_KT):
                            nc.tensor.matmul(
                                pf[:, ht:ht + 1],
                                wr[:, kt, ht * P:(ht + 1) * P],
                                ctxT[:, kt:kt + 1],
                                start=(kt == 0), stop=(kt == N_KT - 1))
                    fin = small_pool.tile([P, N_HT], f32, tag="fin", name="fin")
                    nc.vector.tensor_copy(fin[:], pf[:])
                    nc.sync.dma_start(
                        ctx_d.ap()[b, 0].rearrange("(t p) -> p t", p=P),
                        fin[:])

                cov = None
                ctx_ps = None
                expT_all = None
                pending_epi = None
                for i, (b, lc) in enumerate(chunks):
                    if lc == 0:
                        cov_f = small_pool.tile([1, L], f32, tag="covf", bufs=1,
                                                name="cov_f")
                        nc.sync.dma_start(cov_f[:],
                                          cov_d.ap()[b, :, 0].unsqueeze(0))
                        cov = small_pool.tile([1, L], bf16, tag="cov", bufs=1,
                                              name="cov")
                        nc.vector.tensor_copy(cov[:], cov_f[:])
                        expT_all = small_pool.tile([P, 4 * N_LC], bf16,
                                                   tag="expTall", name="expT_all")
                    # prefetch chunk i+2 (deep DMA pipeline); transpose chunk
                    # i+1 interleaved with this chunk's main matmuls
                    nxt = chunks[i + 1] if i + 1 < len(chunks) else None
                    if i + 2 < len(chunks):
                        loaded[i + 2] = load_keys_chunk(*chunks[i + 2])
                    if nxt is not None:
                        keys_nat_nxt = loaded.pop(i + 1)
                        keysT_nxt = kT_pool.tile([P, N_KT, LCHUNK], bf16,
                                                 tag="kT", name="keysT")
                    # hT[j-part, jt, l] = tanh(W1k^T keysT + cov*w1c + qb)
                    hT = h_pool.tile([P, N_JT, LCHUNK], bf16, tag="hT", name="hT")
                    for jt in range(N_JT):
                        ph = h_psum.tile([P, LCHUNK], f32, tag="h", name="ph")
                        for kt in range(N_KT):
                            nc.tensor.matmul(
                                ph[:], w1k[kt][:, jt * P:(jt + 1) * P],
                                keysT_cur[:, kt], start=(kt == 0), stop=False)
                        nc.tensor.matmul(
                            ph[:], w1c[:, jt * P:(jt + 1) * P],
                            cov[:, lc * LCHUNK:(lc + 1) * LCHUNK],
                            start=False, stop=True)
                        nc.scalar.activation(hT[:, jt], ph[:], AF.Tanh,
                                             bias=qbT[:, jt, b:b + 1])
                        if nxt is not None and jt in (3, 7):
                            for g in range(4 * (jt // 4), 4 * (jt // 4) + 4):
                                emit_transpose_group(keys_nat_nxt, keysT_nxt, g)

                    # previous batch's epilogue: emitted here so its serial
                    # reduce/scale chain overlaps this mains block
                    if pending_epi is not None:
                        emit_epilogue(*pending_epi)
                        pending_epi = None
                    if lc == 0:
                        ctx_ps = [ctx_psum.tile([1, LCHUNK], f32, tag="ctx",
                                                name=f"ctx{hf}")
                                  for hf in range(2)]
                    # scores for the chunk: (1, LCHUNK)
                    ps_s = s_psum.tile([1, LCHUNK], f32, tag="s", name="ps_s")
                    for jt in range(N_JT):
                        nc.tensor.matmul(ps_s[:], W2T[:, jt:jt + 1],
                                         hT[:, jt],
                                         start=(jt == 0), stop=(jt == N_JT - 1))
                    # exp (scores bounded by ||W2||_1: no max needed) + partial sum
                    exp_sb = small_pool.tile([1, LCHUNK], f32, tag="exps",
                                             name="exp_sb")
                    nc.scalar.activation(exp_sb[:], ps_s[:], AF.Exp,
                                         accum_out=sums[:, b, lc:lc + 1])
                    # transpose exp chunk onto partitions (K=1 fp32 matmuls)
                    pse = sm_psum.tile([P, N_LC], f32, tag="small", name="pse")
                    for lt in range(N_LC):
                        nc.tensor.matmul(
                            pse[:, lt:lt + 1],
                            exp_sb[:, lt * P:(lt + 1) * P],
                            ones1[:], start=True, stop=True)
                    nc.vector.tensor_copy(
                        expT_all[:, lc * N_LC:(lc + 1) * N_LC], pse[:])
                    # pass B: ctx_unnorm[k] += exp[l] * keys[l, k]
                    for lt in range(N_LC):
                        for hf in range(2):
                            nc.tensor.matmul(
                                ctx_ps[hf][:],
                                expT_all[:, lc * N_LC + lt:lc * N_LC + lt + 1],
                                keys_nat_cur[lt][:, hf * LCHUNK:(hf + 1) * LCHUNK],
                                start=(lc == 0 and lt == 0),
                                stop=(lc == N_LC - 1 and lt == N_LC - 1))
                    if nxt is not None:
                        keys_nat_cur = keys_nat_nxt
                        keysT_cur = keysT_nxt
                    if lc == N_LC - 1:
                        pending_epi = (b, ctx_ps, expT_all)
                if pending_epi is not None:
                    emit_epilogue(*pending_epi)
                    pending_epi = None

    nc.compile()
    return nc


def _get_nc(repeat=1):
    key = ("nc", repeat)
    if key not in _CACHE:
        _CACHE[key] = _build(repeat)
    return _CACHE[key]


def kernel(query, keys, coverage, W1, b1, W2, Wr):
    from concourse import bass_utils

    nc = _get_nc()
    query = np.ascontiguousarray(query, dtype=np.float32)
    keys = np.ascontiguousarray(keys, dtype=np.float32)
    coverage = np.ascontiguousarray(coverage, dtype=np.float32)
    W1 = np.ascontiguousarray(W1, dtype=np.float32)
    b1 = np.ascontiguousarray(b1, dtype=np.float32)
    W2 = np.ascontiguousarray(W2, dtype=np.float32)
    Wr = np.ascontiguousarray(Wr, dtype=np.float32)

    in_maps = []
    for c in range(NCORES):
        s = slice(c * B_LOC, (c + 1) * B_LOC)
        in_maps.append({
            "query": query[s], "keys": keys[s], "coverage": coverage[s],
            "W1": W1, "b1": b1, "W2": W2, "Wr": Wr,
        })
    res = bass_utils.run_bass_kernel_spmd(nc, in_maps, core_ids=list(range(NCORES)),
                                          trace=PROFILE)
    global LAST_RESULT
    LAST_RESULT = res
    context = np.concatenate([res.results[c]["context"] for c in range(NCORES)], axis=0)
    attention = np.concatenate([res.results[c]["attention"] for c in range(NCORES)], axis=0)
    return context, attention


# revision 32
# speedup vs baseline: 1.9393x; 1.0012x over previous
"""Bahdanau-style additive attention with coverage, on 8 trn2 NeuronCores.

Math (per batch b):
  qb[j]    = sum_k q[k] * W1[k, j] + b1[j]                  (k in [0, 2H))
  pre[l,j] = sum_k keys[l,k] * W1[2H+k, j] + cov[l]*W1[4H, j] + qb[j]
  h        = tanh(pre)
  scores   = h @ W2                   (L,)
  attn     = softmax(scores)          (no max-subtraction: |scores| <= ||W2||_1)
  ctx_keys = attn^T @ keys            (2H,)   [reassociated: attn^T(keys@Wr) = (attn^T keys)@Wr]
  context  = ctx_keys @ Wr            (H,)

Sharding: data-parallel over batch B=16 -> 2 batches per core, params replicated.
Matmuls run as float32r (TF32-like, 1 cyc/row at N=512); K=1 reshape matmuls
stay plain fp32 (fp32r has PSUM destination-pattern restrictions).
"""

import numpy as np

# Problem constants (hardcoded per harness contract).
B = 16
L = 2048
H = 512
KD = 2 * H  # 1024: feature dim of q/keys, also MLP hidden dim
NCORES = 8
B_LOC = B // NCORES  # 2
P = 128
LCHUNK = 512
N_LC = L // LCHUNK  # 4
N_KT = KD // P  # 8 k-tiles (contraction)
N_JT = KD // P  # 8 j-tiles (hidden)
N_HT = H // P  # 4 h-tiles (output)

_CACHE = {}
PROFILE = False  # set True (e.g. from test.py) to capture an NTFF trace
LAST_RESULT = None


def _build(repeat=1):
    import concourse.bass as bass
    import concourse.mybir as mybir
    import concourse.tile as tile
    from concourse import bacc
    from concourse.masks import make_identity

    f32 = mybir.dt.float32
    bf16 = mybir.dt.bfloat16
    f32r = mybir.dt.float32r
    AF = mybir.ActivationFunctionType

    def r(ap):  # view fp32 data as fp32r for reduced-precision matmul
        return ap.bitcast(f32r)

    nc = bacc.Bacc("TRN2", target_bir_lowering=False, debug=False,
                   num_devices=NCORES)

    query_d = nc.dram_tensor("query", (B_LOC, 1, KD), f32, kind="ExternalInput")
    keys_d = nc.dram_tensor("keys", (B_LOC, L, KD), f32, kind="ExternalInput")
    cov_d = nc.dram_tensor("coverage", (B_LOC, L, 1), f32, kind="ExternalInput")
    W1_d = nc.dram_tensor("W1", (2 * KD + 1, KD), f32, kind="ExternalInput")
    b1_d = nc.dram_tensor("b1", (KD,), f32, kind="ExternalInput")
    W2_d = nc.dram_tensor("W2", (KD, 1), f32, kind="ExternalInput")
    Wr_d = nc.dram_tensor("Wr", (KD, H), f32, kind="ExternalInput")
    ctx_d = nc.dram_tensor("context", (B_LOC, 1, H), f32, kind="ExternalOutput")
    attn_d = nc.dram_tensor("attention", (B_LOC, L, 1), f32, kind="ExternalOutput")

    with tile.TileContext(nc) as tc:
        with tc.tile_pool(name="singles", bufs=1) as singles, \
             tc.tile_pool(name="knat", bufs=4) as knat_pool, \
             tc.tile_pool(name="kbf", bufs=12) as kbf_pool, \
             tc.tile_pool(name="kT", bufs=2) as kT_pool, \
             tc.tile_pool(name="hT", bufs=2) as h_pool, \
             tc.tile_pool(name="small_sb", bufs=2) as small_pool, \
             tc.tile_pool(name="w1q", bufs=4) as w1q_pool, \
             tc.tile_pool(name="tp_psum", bufs=2, space="PSUM") as tp_psum, \
             tc.tile_pool(name="h_psum", bufs=2, space="PSUM") as h_psum, \
             tc.tile_pool(name="s_psum", bufs=1, space="PSUM") as s_psum, \
             tc.tile_pool(name="sm_psum", bufs=1, space="PSUM") as sm_psum, \
             tc.tile_pool(name="ctx_psum", bufs=2, space="PSUM") as ctx_psum:

            def load_keys_chunk(b, lc):
                # one tile per 128-row slab: fine-grained DMA deps so each
                # transpose group can start as soon as its slab lands.
                # fp32 slab is cast to bf16 right away (matmul operand).
                tiles = []
                for lt in range(N_LC):
                    t = knat_pool.tile([P, KD], f32, tag="knat",
                                       name="keys_nat")
                    nc.sync.dma_start(
                        t[:],
                        keys_d.ap()[b, lc * LCHUNK + lt * P:
                                    lc * LCHUNK + (lt + 1) * P, :])
                    tb = kbf_pool.tile([P, KD], bf16, tag="kbf",
                                       name="keys_bf")
                    nc.vector.tensor_copy(tb[:], t[:])
                    tiles.append(tb)
                return tiles

            # Chunk (0,0) keys DMA issued before anything else so the PE
            # pipeline head (transposes) is never starved behind param loads.
            first_keys = load_keys_chunk(0, 0)

            ident_f = singles.tile([P, P], f32)
            make_identity(nc, ident_f[:])
            ident_b = singles.tile([P, P], bf16)  # 0/1 exact in bf16
            nc.vector.tensor_copy(ident_b[:], ident_f[:])
            ones1 = singles.tile([1, 1], f32)
            nc.vector.memset(ones1[:], 1.0)
            ones_row = singles.tile([1, P], f32)
            nc.vector.memset(ones_row[:], 1.0)

            # ~8us of dense dummy matmuls (bf16 N=512 streams, high PE duty):
            # pulls the PE HAM clock-gate to 8/8 while the head DMAs stream,
            # so the real matmuls start at 2.4GHz instead of 1.2.
            warm_sb = singles.tile([P, LCHUNK], bf16, name="warm_sb")
            nc.vector.memset(warm_sb[:], 0.125)
            ps_warm = s_psum.tile([P, LCHUNK], f32, tag="s", name="ps_warm")
            for _w in range(40):
                nc.tensor.matmul(ps_warm[:], ident_b[:], warm_sb[:],
                                 start=True, stop=True)

            # W1 keys-part in bf16: DMA fp32 staged through the knat pool,
            # cast once on the DVE. (DMAs issued inside rep 0, after the W1q
            # stream, so qb is never starved behind them.)
            w1k = []
            for kt in range(N_KT):
                w1k_t = singles.tile([P, KD], bf16, name=f"w1k{kt}")
                w1k.append(w1k_t)
            w1c_f = singles.tile([1, KD], f32)
            nc.sync.dma_start(w1c_f[:], W1_d.ap()[2 * KD:2 * KD + 1, :])
            w1c = singles.tile([1, KD], bf16)  # W1[2KD] (coverage row)
            nc.vector.tensor_copy(w1c[:], w1c_f[:])

            # b1, W2 transposed onto partitions: [p, t] = v[t*P + p]
            b1T = singles.tile([P, N_JT], f32)
            nc.sync.dma_start(b1T[:], b1_d.ap().rearrange("(t p) -> p t", p=P))
            W2T_f = singles.tile([P, N_JT], f32)
            nc.sync.dma_start(W2T_f[:], W2_d.ap().rearrange("(t p) o -> p (t o)", p=P))
            W2T = singles.tile([P, N_JT], bf16)
            nc.vector.tensor_copy(W2T[:], W2T_f[:])

            # q transposed: q_sb[p, b, kt] = query[b, 0, kt*P + p]
            q_sb = singles.tile([P, B_LOC, N_KT], f32)
            for b in range(B_LOC):
                nc.sync.dma_start(
                    r(q_sb[:, b]),
                    r(query_d.ap()[b, 0].rearrange("(t p) -> p t", p=P)))

            for _rep in range(repeat):
                def emit_qb():
                    # qbT[j-part, jt, b] = q @ W1q + b1. W1q streams through a
                    # small pool; all 8 jt groups accumulate into one PSUM bank.
                    qbT_ = singles.tile([P, N_JT, B_LOC], f32, name="qbT")
                    ps_qb = sm_psum.tile([P, N_JT * B_LOC], f32, tag="small",
                                         name="ps_qb")
                    for kt in range(N_KT):
                        w1q_c = w1q_pool.tile([P, KD], f32, tag="w1q",
                                              name="w1q_c")
                        for hf in range(2):
                            nc.sync.dma_start(
                                r(w1q_c[:, hf * LCHUNK:(hf + 1) * LCHUNK]),
                                r(W1_d.ap()[kt * P:(kt + 1) * P,
                                            hf * LCHUNK:(hf + 1) * LCHUNK]))
                        for jt in range(N_JT):
                            nc.tensor.matmul(
                                ps_qb[:, jt * B_LOC:(jt + 1) * B_LOC],
                                r(w1q_c[:, jt * P:(jt + 1) * P]),
                                r(q_sb[:, :, kt]),
                                start=(kt == 0 and jt == 0),
                                stop=(kt == N_KT - 1 and jt == N_JT - 1))
                        # keep the PE HAM-warm while the W1q stream trickles in
                        for _w in range(3):
                            nc.tensor.matmul(ps_warm[:], ident_b[:],
                                             warm_sb[:], start=True, stop=True)
                    for jt in range(N_JT):
                        nc.vector.tensor_scalar_add(
                            qbT_[:, jt], ps_qb[:, jt * B_LOC:(jt + 1) * B_LOC],
                            b1T[:, jt:jt + 1])
                    return qbT_

                def emit_transpose_group(keys_nat, keysT, g):
                    # group g handles slab lt = g//2, kt-half kh = g%2:
                    # 4 bf16 PE transposes into one PSUM bank + copy out.
                    # Transpose-mode doesn't register as PE activity for the
                    # HAM clock-gate, so a real (dummy) matmul follows each
                    # group: the gate never sees a full idle window and the
                    # PE stays at 2.4GHz.
                    lt, kh = g // 2, g % 2
                    pst = tp_psum.tile([P, LCHUNK], bf16, tag="tp", name="pst")
                    for j in range(4):
                        kt = kh * 4 + j
                        nc.tensor.transpose(
                            pst[:, j * P:(j + 1) * P],
                            keys_nat[lt][:, kt * P:(kt + 1) * P],
                            ident_b[:])
                    nc.vector.tensor_copy(
                        keysT[:, kh * 4:(kh + 1) * 4, lt * P:(lt + 1) * P],
                        pst[:].rearrange("p (j l) -> p j l", j=4))

                # softmax state
                sums = singles.tile([1, B_LOC, N_LC], f32, name="sums")
                rsum = singles.tile([1, B_LOC], f32, name="rsum")

                chunks = [(b, lc) for b in range(B_LOC) for lc in range(N_LC)]

                # ---- prologue: qb (its W1q DMAs go out first among params,
                # PE meanwhile runs the warmup), then transpose chunk 0 ----
                qbT = emit_qb()
                if _rep == 0:
                    for kt in range(N_KT):
                        w1k_f = knat_pool.tile([P, KD], f32, tag="knat",
                                               name="w1k_f")
                        nc.sync.dma_start(
                            w1k_f[:],
                            W1_d.ap()[KD + kt * P:KD + (kt + 1) * P, :])
                        nc.vector.tensor_copy(w1k[kt][:], w1k_f[:])
                    keys_nat_cur = first_keys
                else:
                    keys_nat_cur = load_keys_chunk(0, 0)
                keysT_cur = kT_pool.tile([P, N_KT, LCHUNK], bf16, tag="kT",
                                         name="keysT")
                for g in range(8):
                    emit_transpose_group(keys_nat_cur, keysT_cur, g)
                loaded = {1: load_keys_chunk(*[(b, lc) for b in range(B_LOC)
                                               for lc in range(N_LC)][1])}
                wr = singles.tile([P, N_KT, H], f32, name="wr")
                for kt in range(N_KT):
                    nc.sync.dma_start(wr[:, kt],
                                      Wr_d.ap()[kt * P:(kt + 1) * P, :])

                def emit_epilogue(b, ctx_ps, expT_all):
                    nc.vector.tensor_reduce(rsum[:, b:b + 1], sums[:, b],
                                            axis=mybir.AxisListType.X,
                                            op=mybir.AluOpType.add)
                    nc.vector.reciprocal(rsum[:, b:b + 1], rsum[:, b:b + 1])
                    # broadcast 1/sum over partitions via a K=1 matmul
                    ps_rb = sm_psum.tile([P, 1], f32, tag="small", name="ps_rb")
                    nc.tensor.matmul(ps_rb[:], ones_row[:], rsum[:, b:b + 1],
                                     start=True, stop=True)
                    rsum_b = small_pool.tile([P, 1], f32, tag="rsum_b",
                                             name="rsum_b")
                    nc.vector.tensor_copy(rsum_b[:], ps_rb[:])
                    # attention out: normalize the partition-major exp tile,
                    # transpose back to l-major on the PE, then one DMA
                    attn_n = small_pool.tile([P, 4 * N_LC], f32, tag="attn_n",
                                             name="attn_n")
                    nc.vector.tensor_scalar_mul(attn_n[:], expT_all[:],
                                                rsum_b[:])
                    ps_at = tp_psum.tile([4 * N_LC, P], f32, tag="tp",
                                         name="ps_at")
                    nc.tensor.transpose(ps_at[:], attn_n[:], ident_f[:])
                    attn_sb = small_pool.tile([4 * N_LC, P], f32, tag="attn_sb",
                                              name="attn_sb")
                    nc.vector.tensor_copy(attn_sb[:], ps_at[:])
                    nc.sync.dma_start(
                        attn_d.ap()[b, :, 0].rearrange("(t l) -> t l", l=P),
                        attn_sb[:])
                    # ctx_keys row, normalized
                    ctx_row = small_pool.tile([1, KD], f32, tag="ctxrow",
                                              bufs=1, name="ctx_row")
                    for hf in range(2):
                        nc.scalar.mul(ctx_row[:, hf * LCHUNK:(hf + 1) * LCHUNK],
                                      ctx_ps[hf][:], rsum[:, b:b + 1])
                    # transpose ctx_keys onto partitions (K=1 fp32 matmuls)
                    psc = sm_psum.tile([P, N_KT], f32, tag="small", name="psc")
                    for kt in range(N_KT):
                        nc.tensor.matmul(psc[:, kt:kt + 1],
                                         ctx_row[:, kt * P:(kt + 1) * P],
                                         ones1[:], start=True, stop=True)
                    ctxT = small_pool.tile([P, N_KT], f32, tag="ctxT",
                                           bufs=2, name="ctxT")
                    nc.vector.tensor_copy(ctxT[:], psc[:])
                    # context[b] = ctx_keys @ Wr (plain fp32: N=1 rhs)
                    pf = sm_psum.tile([P, N_HT], f32, tag="small", name="pf")
                    for ht in range(N_HT):
                        for kt in range(N_KT):
                            nc.tensor.matmul(
                                pf[:, ht:ht + 1],
                                wr[:, kt, ht * P:(ht + 1) * P],
                                ctxT[:, kt:kt + 1],
                                start=(kt == 0), stop=(kt == N_KT - 1))
                    fin = small_pool.tile([P, N_HT], f32, tag="fin", name="fin")
                    nc.vector.tensor_copy(fin[:], pf[:])
                    nc.sync.dma_start(
                        ctx_d.ap()[b, 0].rearrange("(t p) -> p t", p=P),
                        fin[:])

                cov = None
                ctx_ps = None
                expT_all = None
                pending_epi = None
                for i, (b, lc) in enumerate(chunks):
                    if lc == 0:
                        cov_f = small_pool.tile([1, L], f32, tag="covf", bufs=1,
                                                name="cov_f")
                        nc.sync.dma_start(cov_f[:],
                                          cov_d.ap()[b, :, 0].unsqueeze(0))
                        cov = small_pool.tile([1, L], bf16, tag="cov", bufs=1,
                                              name="cov")
                        nc.vector.tensor_copy(cov[:], cov_f[:])
                        expT_all = small_pool.tile([P, 4 * N_LC], bf16,
                                                   tag="expTall", name="expT_all")
                    # prefetch chunk i+2 (deep DMA pipeline); transpose chunk
                    # i+1 interleaved with this chunk's main matmuls
                    nxt = chunks[i + 1] if i + 1 < len(chunks) else None
                    if i + 2 < len(chunks):
                        loaded[i + 2] = load_keys_chunk(*chunks[i + 2])
                    if nxt is not None:
                        keys_nat_nxt = loaded.pop(i + 1)
                        keysT_nxt = kT_pool.tile([P, N_KT, LCHUNK], bf16,
                                                 tag="kT", name="keysT")
                    # hT[j-part, jt, l] = tanh(W1k^T keysT + cov*w1c + qb)
                    hT = h_pool.tile([P, N_JT, LCHUNK], bf16, tag="hT", name="hT")
                    for jt in range(N_JT):
                        ph = h_psum.tile([P, LCHUNK], f32, tag="h", name="ph")
                        for kt in range(N_KT):
                            nc.tensor.matmul(
                                ph[:], w1k[kt][:, jt * P:(jt + 1) * P],
                                keysT_cur[:, kt], start=(kt == 0), stop=False)
                        nc.tensor.matmul(
                            ph[:], w1c[:, jt * P:(jt + 1) * P],
                            cov[:, lc * LCHUNK:(lc + 1) * LCHUNK],
                            start=False, stop=True)
                        nc.scalar.activation(hT[:, jt], ph[:], AF.Tanh,
                                             bias=qbT[:, jt, b:b + 1])
                        if nxt is not None and jt in (3, 7):
                            for g in range(4 * (jt // 4), 4 * (jt // 4) + 4):
                                emit_transpose_group(keys_nat_nxt, keysT_nxt, g)

                    # previous batch's epilogue: emitted here so its serial
                    # reduce/scale chain overlaps this mains block
                    if pending_epi is not None:
                        emit_epilogue(*pending_epi)
                        pending_epi = None
                    if lc == 0:
                        ctx_ps = [ctx_psum.tile([1, LCHUNK], f32, tag="ctx",
                                                name=f"ctx{hf}")
                                  for hf in range(2)]
                    # scores for the chunk: (1, LCHUNK)
                    ps_s = s_psum.tile([1, LCHUNK], f32, tag="s", name="ps_s")
                    for jt in range(N_JT):
                        nc.tensor.matmul(ps_s[:], W2T[:, jt:jt + 1],
                                         hT[:, jt],
                                         start=(jt == 0), stop=(jt == N_JT - 1))
                    # exp (scores bounded by ||W2||_1: no max needed) + partial sum
                    exp_sb = small_pool.tile([1, LCHUNK], f32, tag="exps",
                                             name="exp_sb")
                    nc.scalar.activation(exp_sb[:], ps_s[:], AF.Exp,
                                         accum_out=sums[:, b, lc:lc + 1])
                    # transpose exp chunk onto partitions (K=1 fp32 matmuls)
                    pse = sm_psum.tile([P, N_LC], f32, tag="small", name="pse")
                    for lt in range(N_LC):
                        nc.tensor.matmul(
                            pse[:, lt:lt + 1],
                            exp_sb[:, lt * P:(lt + 1) * P],
                            ones1[:], start=True, stop=True)
                    nc.vector.tensor_copy(
                        expT_all[:, lc * N_LC:(lc + 1) * N_LC], pse[:])
                    # pass B: ctx_unnorm[k] += exp[l] * keys[l, k]
                    for lt in range(N_LC):
                        for hf in range(2):
                            nc.tensor.matmul(
                                ctx_ps[hf][:],
                                expT_all[:, lc * N_LC + lt:lc * N_LC + lt + 1],
                                keys_nat_cur[lt][:, hf * LCHUNK:(hf + 1) * LCHUNK],
                                start=(lc == 0 and lt == 0),
                                stop=(lc == N_LC - 1 and lt == N_LC - 1))
                    if nxt is not None:
                        keys_nat_cur = keys_nat_nxt
                        keysT_cur = keysT_nxt
                    if lc == N_LC - 1:
                        pending_epi = (b, ctx_ps, expT_all)
                if pending_epi is not None:
                    emit_epilogue(*pending_epi)
                    pending_epi = None

    nc.compile()
    return nc


def _get_nc(repeat=1):
    key = ("nc", repeat)
    if key not in _CACHE:
        _CACHE[key] = _build(repeat)
    return _CACHE[key]


def kernel(query, keys, coverage, W1, b1, W2, Wr):
    from concourse import bass_utils

    nc = _get_nc()
    query = np.ascontiguousarray(query, dtype=np.float32)
    keys = np.ascontiguousarray(keys, dtype=np.float32)
    coverage = np.ascontiguousarray(coverage, dtype=np.float32)
    W1 = np.ascontiguousarray(W1, dtype=np.float32)
    b1 = np.ascontiguousarray(b1, dtype=np.float32)
    W2 = np.ascontiguousarray(W2, dtype=np.float32)
    Wr = np.ascontiguousarray(Wr, dtype=np.float32)

    in_maps = []
    for c in range(NCORES):
        s = slice(c * B_LOC, (c + 1) * B_LOC)
        in_maps.append({
            "query": query[s], "keys": keys[s], "coverage": coverage[s],
            "W1": W1, "b1": b1, "W2": W2, "Wr": Wr,
        })
    res = bass_utils.run_bass_kernel_spmd(nc, in_maps, core_ids=list(range(NCORES)),
                                          trace=PROFILE)
    global LAST_RESULT
    LAST_RESULT = res
    context = np.concatenate([res.results[c]["context"] for c in range(NCORES)], axis=0)
    attention = np.concatenate([res.results[c]["attention"] for c in range(NCORES)], axis=0)
    return context, attention
